# revision 14
# baseline (speedup 1.0000x reference)
"""Exponential smoothing (per-channel EMA over time) on 8 Trainium2 cores.

  s_0 = x_0 ; s_t = a * x_t + (1 - a) * s_{t-1},  a = sigmoid(alpha)  (per channel)

Full shapes: x (16, 4096, 512) f32, alpha (1, 1, 512) f32 -> out (16, 4096, 512).

Design (trace-driven):
  * All HBM I/O is fp16, time-major per core, with each time chunk stored as
    blocked halves [even timesteps | odd timesteps]: the host preps the
    layout and upcasts/interleaves the result (pure layout transforms; the
    2e-2 global-rel-err budget dwarfs fp16's 2^-11 and the EMA is a convex
    combination, so input rounding does not amplify). This halves DMA bytes,
    removes on-device transposes, and keeps operands contiguous. Per-channel
    parameter transforms (a, w, w^2, 1/a -- 512 elements) are
    host-precomputed into one (128, 16) f32 tile.
  * The hardware scan (TensorTensorScanArith, vector engine only) costs
    ~2.13 ns/elem/lane regardless of dtype -- the critical resource. The
    kernel scans ONLY the odd timesteps (an EMA with decay w^2 over
    combined inputs g_i = w*x_{2i} + x_{2i+1}), halving vector-engine work:
      - combine g: tensor engine, diag(w) @ x_even + I @ x_odd into PSUM
        (the scan reads data1 straight from PSUM). Combines run one chunk
        ahead of fills so the PE queue never blocks the next scan.
      - odd scan: r_{2i+1} = w^2 * r_{2i-1} + g_i on the vector engine,
        scanning r = s/a (initial r_{-1} = x_0/a) so raw x is the scan
        input. The scan writes cols 1..nh of a [128, nh+1] tile whose col 0
        holds the initial; the fill's shifted operand is then contiguous.
      - even fill: r_{2i} = w * r_{2i-1} + x_{2i} on the tensor engine
        (diag(w) @ r_shift + I @ x_even into PSUM).
      - evacuate+scale y = a * r: even halves (PSUM) on the scalar engine;
        odd halves (SBUF) split scalar/vector (the packed-fp16 DVE fast
        path is ~0.2 ns/elem; gpsimd elementwise ops crash the Q7 handler).
  * Variable chunk sizes: the first chain ramps 512/512/1024/2048 so the
    first scan starts early, and the last chain tapers 2048/1024/512/512 so
    the final fill+evac+store tail is short.
  * Loads ride the SP hardware-DGE queue, stores the GpSimd software-DGE
    queue; both spread across all 16 DMA engines (~44 us floor for 16.8
    MB/core of traffic).
"""

from contextlib import ExitStack

import numpy as np

import concourse.tile as tile
from concourse import bacc, mybir
from concourse.bass_utils import run_bass_kernel_spmd
from concourse.masks import make_identity

B, T, D = 16, 4096, 512
NCORES = 8
BL = B // NCORES   # batches per core
P = 128            # partitions
ND = D // P        # channel chunks of 128
MM = 512           # max moving free dim per matmul

FP32 = mybir.dt.float32
FP16 = mybir.dt.float16


def chunk_schedule():
    """Per-chain chunk size lists, then a round-robin flattened order.

    Chains are (b, j) pairs in row-major order. Chain 0 ramps up, the last
    chain tapers down, the rest use 2048-chunks. Returns a flat list of
    (b, j, t0, tc) preserving per-chain order.
    """
    nchain = BL * ND
    sizes = {0: [512, 512, 1024, 2048], nchain - 1: [2048, 1024, 512, 512]}
    chains = []
    for ci in range(nchain):
        b, j = divmod(ci, ND)
        tcs = sizes.get(ci, [2048, 2048])
        t0s = np.concatenate([[0], np.cumsum(tcs)[:-1]]).astype(int)
        chains.append([(b, j, int(t0), int(tc)) for t0, tc in zip(t0s, tcs)])
    flat = []
    pos = 0
    while any(chains):
        for ch in chains:
            if pos < len(ch):
                flat.append(ch[pos])
        pos += 1
        if pos > 8:
            break
    return flat


CHUNKS = chunk_schedule()


def build_program(bl: int = BL, t: int = T) -> bacc.Bacc:
    """Build the per-core Bass program (same NEFF for all 8 cores)."""
    nc = bacc.Bacc(
        "TRN2",
        target_bir_lowering=False,
        debug=False,
        enable_asserts=False,
        num_devices=NCORES,
    )
    x = nc.dram_tensor("xt", (bl, D, t), FP16, kind="ExternalInput").ap()
    # Host-precomputed per-channel coefficients, partition-major:
    # col q*ND + j = quantity q for channel chunk j (q: 0=a, 1=w, 2=w^2, 3=1/a)
    coef = nc.dram_tensor("coef", (P, 4 * ND), FP32, kind="ExternalInput").ap()
    y = nc.dram_tensor("yt", (bl, D, t), FP16, kind="ExternalOutput").ap()

    with tile.TileContext(nc) as tc, ExitStack() as ctx:
        const_pool = ctx.enter_context(tc.tile_pool(name="const", bufs=1))
        x_pool = ctx.enter_context(tc.tile_pool(name="x", bufs=8))
        g_pool = ctx.enter_context(tc.tile_pool(name="g", bufs=3, space="PSUM"))
        rep_pool = ctx.enter_context(tc.tile_pool(name="rep", bufs=1, space="PSUM"))
        r_pool = ctx.enter_context(tc.tile_pool(name="r", bufs=12))
        y_pool = ctx.enter_context(tc.tile_pool(name="y", bufs=5))

        # Identity first: gpsimd builds it while the coef DMA runs.
        ident = const_pool.tile([P, P], FP16)
        make_identity(nc, ident[:])

        coef_sb = const_pool.tile([P, 4 * ND], FP32)
        nc.sync.dma_start(coef_sb[:], coef[:, :])
        a_sb = coef_sb[:, 0 * ND : 1 * ND]
        w_sb = coef_sb[:, 1 * ND : 2 * ND]
        w2_sb = coef_sb[:, 2 * ND : 3 * ND]
        inv_a = coef_sb[:, 3 * ND : 4 * ND]

        diag_w = []
        for j in range(ND):
            dw = const_pool.tile([P, P], FP16, tag=f"dw{j}")
            nc.vector.tensor_scalar_mul(dw[:], ident[:], w_sb[:, j : j + 1])
            diag_w.append(dw)

        # Scan data0: w^2 broadcast along time (sliced per chunk size).
        ones = const_pool.tile([P, 1024], FP16)
        nc.vector.memset(ones[:], 1.0)
        w2b = []
        for j in range(ND):
            wt = const_pool.tile([P, 1024], FP16, tag=f"w2b{j}")
            nc.scalar.mul(wt[:], ones[:], w2_sb[:, j : j + 1])
            w2b.append(wt)

        n = len(CHUNKS)
        xcs, gps = [None] * n, [None] * n
        r_prev = [[None] * ND for _ in range(bl)]  # (tile, nh) per chain

        def load_and_combine(c):
            b, j, t0, tcs = CHUNKS[c]
            nh = tcs // 2
            xc = x_pool.tile([P, tcs], FP16, tag="x", name=f"x{c}")
            nc.sync.dma_start(xc[:], x[b, j * P : (j + 1) * P, t0 : t0 + tcs])
            gp = g_pool.tile([P, nh], FP32, tag="g", name=f"g{c}")
            for h in range((nh + MM - 1) // MM):
                c0, c1 = h * MM, min((h + 1) * MM, nh)
                nc.tensor.matmul(
                    gp[:, c0:c1], diag_w[j][:], xc[:, c0:c1], start=True, stop=False
                )
            for h in range((nh + MM - 1) // MM):
                c0, c1 = h * MM, min((h + 1) * MM, nh)
                nc.tensor.matmul(
                    gp[:, c0:c1], ident[:], xc[:, nh + c0 : nh + c1],
                    start=False, stop=True,
                )
            xcs[c], gps[c] = xc, gp

        load_and_combine(0)
        load_and_combine(1)
        for c in range(n):
            b, j, t0, tcs = CHUNKS[c]
            nh = tcs // 2
            xc, gp = xcs[c], gps[c]

            # ro col 0 = initial r_{-1}; cols 1..nh = scan of odds.
            ro = r_pool.tile([P, nh + 1], FP16, tag="r", name=f"r{c}")
            if t0 == 0:
                nc.vector.tensor_scalar_mul(
                    ro[:, 0:1], xc[:, 0:1], inv_a[:, j : j + 1]
                )
            else:
                pro, pnh = r_prev[b][j]
                nc.vector.tensor_copy(ro[:, 0:1], pro[:, pnh : pnh + 1])
            nc.vector.tensor_tensor_scan(
                ro[:, 1 : nh + 1],
                w2b[j][:, 0:nh],
                gp[:],
                ro[:, 0:1],
                mybir.AluOpType.mult,
                mybir.AluOpType.add,
            )
            r_prev[b][j] = (ro, nh)

            # Next chunk's combine goes on the PE queue BEFORE this fill.
            if c + 2 < n:
                load_and_combine(c + 2)

            # r_{2i} = w * r_{2i-1} + x_{2i}: diag(w) @ ro_shift + I @ x_even
            rep = rep_pool.tile([P, nh], FP32, tag="rep", name=f"rep{c}")
            for h in range((nh + MM - 1) // MM):
                c0, c1 = h * MM, min((h + 1) * MM, nh)
                nc.tensor.matmul(
                    rep[:, c0:c1], diag_w[j][:], ro[:, c0:c1], start=True, stop=False
                )
            for h in range((nh + MM - 1) // MM):
                c0, c1 = h * MM, min((h + 1) * MM, nh)
                nc.tensor.matmul(
                    rep[:, c0:c1], ident[:], xc[:, c0:c1], start=False, stop=True
                )

            # y = a * r, blocked [evens | odds]; host de-interleaves.
            yc = y_pool.tile([P, tcs], FP16, tag="y", name=f"y{c}")
            nc.scalar.mul(yc[:, 0:nh], rep[:], a_sb[:, j : j + 1])
            if b == 0:
                nc.scalar.mul(yc[:, nh:tcs], ro[:, 1 : nh + 1], a_sb[:, j : j + 1])
            else:
                nc.vector.tensor_scalar_mul(
                    yc[:, nh:tcs], ro[:, 1 : nh + 1], a_sb[:, j : j + 1]
                )
            nc.gpsimd.dma_start(y[b, j * P : (j + 1) * P, t0 : t0 + tcs], yc[:])

    nc.compile()
    return nc


_prog = None


def _get_prog():
    global _prog
    if _prog is None:
        _prog = build_program()
    return _prog


def make_coef(alpha):
    """Precompute per-channel (a, w, w^2, 1/a) packed as (128, 4*ND) f32."""
    al = np.asarray(alpha, dtype=np.float64).reshape(D)
    a = 1.0 / (1.0 + np.exp(-al))
    w = 1.0 - a
    quants = [a, w, w * w, 1.0 / a]
    out = np.empty((P, 4 * ND), dtype=np.float32)
    for q, v in enumerate(quants):
        # channel d = j*128 + p -> column q*ND + j, row p
        out[:, q * ND : (q + 1) * ND] = v.reshape(ND, P).T
    return out


def _block_inplace(arr):
    """arr (B, D, T) fp16, natural time order -> per-chunk [evens | odds]."""
    out = np.empty_like(arr)
    for b, j, t0, tcs in CHUNKS:
        # b is the per-core batch index; apply to every core's batch b
        for core in range(NCORES):
            gb = core * BL + b
            out[gb, j * P : (j + 1) * P, t0 : t0 + tcs] = np.concatenate(
                [
                    arr[gb, j * P : (j + 1) * P, t0 : t0 + tcs][..., 0::2],
                    arr[gb, j * P : (j + 1) * P, t0 : t0 + tcs][..., 1::2],
                ],
                axis=-1,
            )
    return out


def _unblock_inplace(arr):
    """arr (B, D, T) fp16, per-chunk [evens | odds] -> natural time order."""
    out = np.empty_like(arr)
    for b, j, t0, tcs in CHUNKS:
        nh = tcs // 2
        for core in range(NCORES):
            gb = core * BL + b
            seg = arr[gb, j * P : (j + 1) * P, t0 : t0 + tcs]
            nat = np.empty_like(seg)
            nat[..., 0::2] = seg[..., 0:nh]
            nat[..., 1::2] = seg[..., nh:tcs]
            out[gb, j * P : (j + 1) * P, t0 : t0 + tcs] = nat
    return out


def make_in_maps(x, alpha):
    """Per-core inputs: blocked-even/odd time-major fp16 x + coef tile."""
    x = np.asarray(x)
    alpha = np.asarray(alpha)
    assert x.shape == (B, T, D) and alpha.shape == (1, 1, D)
    coef = make_coef(alpha)
    xt = np.ascontiguousarray(x.transpose(0, 2, 1)).astype(np.float16)  # (B, D, T)
    xb = _block_inplace(xt)
    return [
        {"xt": np.ascontiguousarray(xb[i * BL : (i + 1) * BL]), "coef": coef}
        for i in range(NCORES)
    ]


def gather(results):
    """(NCORES, BL, D, T) fp16 blocked shards -> (B, T, D) f32."""
    yt = np.concatenate([r["yt"] for r in results], axis=0)  # (B, D, T) blocked
    nat = _unblock_inplace(yt)
    return np.ascontiguousarray(nat.transpose(0, 2, 1)).astype(np.float32)


def kernel(x, alpha):
    res = run_bass_kernel_spmd(
        _get_prog(), make_in_maps(x, alpha), core_ids=list(range(NCORES))
    )
    return gather(res.results)


# revision 15
# speedup vs baseline: 1.0778x; 1.0778x over previous
"""Exponential smoothing (per-channel EMA over time) on 8 Trainium2 cores.

  s_0 = x_0 ; s_t = a * x_t + (1 - a) * s_{t-1},  a = sigmoid(alpha)  (per channel)

Full shapes: x (16, 4096, 512) f32, alpha (1, 1, 512) f32 -> out (16, 4096, 512).

Design (trace-driven; 108.7 us baseline -> 70.6 us):
  * All HBM I/O is fp16, time-major per core: host preps (D, T) fp16 shards
    and upcasts the fp16 result (2e-2 global-rel-err budget vs fp16's 2^-11;
    the EMA is a convex combination, so input rounding does not amplify).
    Halves DMA bytes (~44 us floor for 16.8 MB/core) and puts channels on
    partitions / time on the free axis with no on-device transposes. The
    per-channel parameter transforms (a = sigmoid(alpha), w, w^2, 1/a --
    512 elements) are host-precomputed into one contiguous (128, 16) f32
    tile (a scattered alpha rearrange DMA costs ~784 4-byte packets).
  * The hardware scan (TensorTensorScanArith, vector engine only -- it does
    not compile for gpsimd) costs ~2.13 ns/elem/lane regardless of dtype --
    the critical resource. The kernel scans ONLY the odd timesteps (an EMA
    with decay w^2 over combined inputs g_i = w*x_{2i} + x_{2i+1}), halving
    vector-engine scan work to ~37 us:
      - combine g: tensor engine, diag(w) @ x_even + I @ x_odd accumulated
        into PSUM (the scan reads data1 straight from PSUM). Combines are
        issued one chunk ahead of the fill and g triple-buffers, so the PE
        queue never head-of-line blocks the next scan behind a fill.
      - odd scan: r_{2i+1} = w^2 * r_{2i-1} + g_i on the vector engine.
        It scans r = s/a (initial r_{-1} = x_0/a), which makes raw x the
        scan input -- no pre-scale pass. The scan writes cols 1..NH of an
        [128, NH+1] tile whose col 0 holds the initial, so the shifted
        operand the fill needs is a contiguous slice.
      - even fill: r_{2i} = w * r_{2i-1} + x_{2i}, also on the tensor
        engine (diag(w) @ r_shift + I @ x_even into PSUM). Matmuls are
        batched per stationary matrix to limit LDWEIGHTS churn.
      - evacuate+scale y = a * r: even halves (PSUM source) on the scalar
        engine; odd halves (SBUF source) alternate scalar engine / vector
        engine (the packed-fp16 DVE fast path is ~0.2 ns/elem; gpsimd
        elementwise ops crash the Q7 handler -- do not use). y is written
        in blocked half layout [evens | odds] per chunk (contiguous
        writes); the host de-interleaves.
  * Loads ride the SP hardware-DGE queue, stores the GpSimd software-DGE
    queue; both spread across all 16 DMA engines.
"""

from contextlib import ExitStack

import numpy as np

import concourse.tile as tile
from concourse import bacc, mybir
from concourse.bass_utils import run_bass_kernel_spmd
from concourse.masks import make_identity

B, T, D = 16, 4096, 512
NCORES = 8
BL = B // NCORES   # batches per core
P = 128            # partitions
TC = 2048          # time chunk per pipeline step
NH = TC // 2       # odd (= even) timesteps per chunk
ND = D // P        # channel chunks of 128
MM = 512           # max moving free dim per matmul
NTC = T // TC

FP32 = mybir.dt.float32
FP16 = mybir.dt.float16


def build_program(bl: int = BL, t: int = T) -> bacc.Bacc:
    """Build the per-core Bass program (same NEFF for all 8 cores)."""
    ntc = t // TC
    nc = bacc.Bacc(
        "TRN2",
        target_bir_lowering=False,
        debug=False,
        enable_asserts=False,
        num_devices=NCORES,
    )
    x = nc.dram_tensor("xt", (bl, D, t), FP16, kind="ExternalInput").ap()
    # Host-precomputed per-channel coefficients, partition-major:
    # col q*ND + j = quantity q for channel chunk j (q: 0=a, 1=w, 2=w^2, 3=1/a)
    coef = nc.dram_tensor("coef", (P, 4 * ND), FP32, kind="ExternalInput").ap()
    y = nc.dram_tensor("yt", (bl, D, t), FP16, kind="ExternalOutput").ap()

    with tile.TileContext(nc) as tc, ExitStack() as ctx:
        const_pool = ctx.enter_context(tc.tile_pool(name="const", bufs=1))
        x_pool = ctx.enter_context(tc.tile_pool(name="x", bufs=6))
        g_pool = ctx.enter_context(tc.tile_pool(name="g", bufs=3, space="PSUM"))
        rep_pool = ctx.enter_context(tc.tile_pool(name="rep", bufs=1, space="PSUM"))
        r_pool = ctx.enter_context(tc.tile_pool(name="r", bufs=10))
        y_pool = ctx.enter_context(tc.tile_pool(name="y", bufs=4))

        # Identity first: gpsimd builds it while the coef DMA runs.
        ident = const_pool.tile([P, P], FP16)
        make_identity(nc, ident[:])

        coef_sb = const_pool.tile([P, 4 * ND], FP32)
        nc.sync.dma_start(coef_sb[:], coef[:, :])
        a_sb = coef_sb[:, 0 * ND : 1 * ND]
        w_sb = coef_sb[:, 1 * ND : 2 * ND]
        w2_sb = coef_sb[:, 2 * ND : 3 * ND]
        inv_a = coef_sb[:, 3 * ND : 4 * ND]

        diag_w = []
        for j in range(ND):
            dw = const_pool.tile([P, P], FP16, tag=f"dw{j}")
            nc.vector.tensor_scalar_mul(dw[:], ident[:], w_sb[:, j : j + 1])
            diag_w.append(dw)

        # Scan data0: w^2 broadcast along the time axis (full packed operand).
        ones = const_pool.tile([P, NH], FP16)
        nc.vector.memset(ones[:], 1.0)
        w2b = []
        for j in range(ND):
            wt = const_pool.tile([P, NH], FP16, tag=f"w2b{j}")
            nc.scalar.mul(wt[:], ones[:], w2_sb[:, j : j + 1])
            w2b.append(wt)

        # 8 chains (2 batches x 4 channel chunks), ntc chunks each, chunk
        # order: all chunk-0s, then all chunk-1s (carries ready early).
        chunks = [
            (tci, b, j) for tci in range(ntc) for b in range(bl) for j in range(ND)
        ]
        n = len(chunks)
        xcs, gps = [None] * n, [None] * n
        r_prev = [[None] * ND for _ in range(bl)]

        def load_and_combine(c):
            tci, b, j = chunks[c]
            t0 = tci * TC
            xc = x_pool.tile([P, TC], FP16, tag="x", name=f"x{c}")
            nc.sync.dma_start(xc[:], x[b, j * P : (j + 1) * P, t0 : t0 + TC])
            # g_i = w * x_{2i} + x_{2i+1}: per-stationary batched matmuls.
            gp = g_pool.tile([P, NH], FP32, tag="g", name=f"g{c}")
            for h in range(NH // MM):
                c0, c1 = h * MM, (h + 1) * MM
                nc.tensor.matmul(
                    gp[:, c0:c1], diag_w[j][:],
                    xc[:, 2 * c0 : 2 * c1 : 2], start=True, stop=False,
                )
            for h in range(NH // MM):
                c0, c1 = h * MM, (h + 1) * MM
                nc.tensor.matmul(
                    gp[:, c0:c1], ident[:],
                    xc[:, 2 * c0 + 1 : 2 * c1 : 2], start=False, stop=True,
                )
            xcs[c], gps[c] = xc, gp

        load_and_combine(0)
        load_and_combine(1)
        for c in range(n):
            tci, b, j = chunks[c]
            t0 = tci * TC
            xc, gp = xcs[c], gps[c]

            # ro col 0 = initial r_{-1}; cols 1..NH = scan of odds.
            ro = r_pool.tile([P, NH + 1], FP16, tag="r", name=f"r{c}")
            if tci == 0:
                nc.vector.tensor_scalar_mul(
                    ro[:, 0:1], xc[:, 0:1], inv_a[:, j : j + 1]
                )
            else:
                nc.vector.tensor_copy(ro[:, 0:1], r_prev[b][j][:, NH : NH + 1])
            nc.vector.tensor_tensor_scan(
                ro[:, 1 : NH + 1],
                w2b[j][:],
                gp[:],
                ro[:, 0:1],
                mybir.AluOpType.mult,
                mybir.AluOpType.add,
            )
            r_prev[b][j] = ro

            # Next chunk's combine goes on the PE queue BEFORE this fill.
            if c + 2 < n:
                load_and_combine(c + 2)

            # r_{2i} = w * r_{2i-1} + x_{2i}: diag(w) @ ro_shift + I @ x_even
            rep = rep_pool.tile([P, NH], FP32, tag="rep", name=f"rep{c}")
            for h in range(NH // MM):
                c0, c1 = h * MM, (h + 1) * MM
                nc.tensor.matmul(
                    rep[:, c0:c1], diag_w[j][:], ro[:, c0:c1],
                    start=True, stop=False,
                )
            for h in range(NH // MM):
                c0, c1 = h * MM, (h + 1) * MM
                nc.tensor.matmul(
                    rep[:, c0:c1], ident[:],
                    xc[:, 2 * c0 : 2 * c1 : 2], start=False, stop=True,
                )

            # y = a * r, blocked [evens | odds]; host de-interleaves.
            yc = y_pool.tile([P, TC], FP16, tag="y", name=f"y{c}")
            nc.scalar.mul(yc[:, 0:NH], rep[:], a_sb[:, j : j + 1])
            if b == 0:
                nc.scalar.mul(yc[:, NH:TC], ro[:, 1 : NH + 1], a_sb[:, j : j + 1])
            else:
                nc.vector.tensor_scalar_mul(
                    yc[:, NH:TC], ro[:, 1 : NH + 1], a_sb[:, j : j + 1]
                )
            nc.gpsimd.dma_start(y[b, j * P : (j + 1) * P, t0 : t0 + TC], yc[:])

    nc.compile()
    return nc


_prog = None


def _get_prog():
    global _prog
    if _prog is None:
        _prog = build_program()
    return _prog


def make_coef(alpha):
    """Precompute per-channel (a, w, w^2, 1/a) packed as (128, 4*ND) f32."""
    al = np.asarray(alpha, dtype=np.float64).reshape(D)
    a = 1.0 / (1.0 + np.exp(-al))
    w = 1.0 - a
    quants = [a, w, w * w, 1.0 / a]
    out = np.empty((P, 4 * ND), dtype=np.float32)
    for q, v in enumerate(quants):
        # channel d = j*128 + p -> column q*ND + j, row p
        out[:, q * ND : (q + 1) * ND] = v.reshape(ND, P).T
    return out


def make_in_maps(x, alpha):
    """Per-core inputs: time-major fp16 shard of x + replicated coef."""
    x = np.asarray(x)
    alpha = np.asarray(alpha)
    assert x.shape == (B, T, D) and alpha.shape == (1, 1, D)
    coef = make_coef(alpha)
    xt = np.ascontiguousarray(x.transpose(0, 2, 1)).astype(np.float16)  # (B, D, T)
    return [
        {"xt": np.ascontiguousarray(xt[i * BL : (i + 1) * BL]), "coef": coef}
        for i in range(NCORES)
    ]


def gather(results):
    """(NCORES, BL, D, T) fp16 blocked shards -> (B, T, D) f32.

    Per TC chunk the device wrote [NH evens | NH odds]; interleave back.
    """
    yt = np.concatenate([r["yt"] for r in results], axis=0)  # (B, D, T) blocked
    blk = yt.reshape(B, D, NTC, 2, NH)  # [..., 0, :] evens, [..., 1, :] odds
    nat = blk.transpose(0, 1, 2, 4, 3).reshape(B, D, T)  # interleave
    return np.ascontiguousarray(nat.transpose(0, 2, 1)).astype(np.float32)


def kernel(x, alpha):
    res = run_bass_kernel_spmd(
        _get_prog(), make_in_maps(x, alpha), core_ids=list(range(NCORES))
    )
    return gather(res.results)
